# revision 25
# baseline (speedup 1.0000x reference)
"""Trainium2 Bass kernel for 3x3 VALID conv: x[32,128,64,64] * w[256,128,3,3] + bias.

Strategy:
  - Data-parallel over batch: 8 cores x 4 images each; weights/bias replicated.
  - Per core: implicit GEMM. Contraction dim = C_IN = 128 = partition dim.
    For each filter tap (u,v), accumulate
        psum[o, p] += W[c, o; u,v].T @ x[c, p + u*64 + v]
    over the flattened output grid of 62 rows x 64 cols (contiguous moving
    operand; the last 2 cols of each row are invalid and trimmed on host).
  - float32r matmuls (1 cycle/row for free-dim >= 256 vs 4 cycles/row fp32).
  - Critical-path-first DMA: w tap0 + first 17 input rows land first; weights
    stream on the ScalarE HWDGE ring in parallel with inputs on the Sync ring.
  - PSUM -> SBUF evacuation + bias add on VectorE; one output DMA per chunk.
"""

import numpy as np

import concourse.bacc as bacc
import concourse.tile as tile
from concourse import mybir
from concourse.bass_utils import run_bass_kernel_spmd

N_CORES = 8
B_FULL, C_IN, H, W = 32, 128, 64, 64
C_OUT, KH, KW = 256, 3, 3
B_LOC = B_FULL // N_CORES          # images per core
H_OUT, W_OUT = H - KH + 1, W - KW + 1   # 62, 62
N_HALF = C_OUT // 128              # 2 output-channel halves
ROWS_PER_CHUNK = 8                 # 8 out rows x 64 cols = 512 = one PSUM bank
X_PAD = (H_OUT + KH - 1) * W + 128  # padded free size so shifted reads stay in-bounds
X_PIECES = [(0, 656), (656, 1824), (1824, 2976), (2976, H * W)]

_cached = {}


def _build_nc():
    f32 = mybir.dt.float32
    f32r = mybir.dt.float32r
    nc = bacc.Bacc()

    x_d = nc.declare_dram_parameter("x", [B_LOC, C_IN, H, W], f32r, isOutput=False)
    w_d = nc.declare_dram_parameter(
        "w", [C_IN, N_HALF, KH * KW, 128], f32r, isOutput=False
    )
    b_d = nc.declare_dram_parameter("bias_in", [128, N_HALF], f32, isOutput=False)
    y_d = nc.declare_dram_parameter(
        "y", [B_LOC, N_HALF, 128, H_OUT, W], f32, isOutput=True
    )

    n_chunks = (H_OUT + ROWS_PER_CHUNK - 1) // ROWS_PER_CHUNK
    with tile.TileContext(nc) as tc:
        with (
            tc.tile_pool(name="const", bufs=1) as cpool,
            tc.tile_pool(name="xin", bufs=2) as xpool,
            tc.tile_pool(name="out", bufs=4) as opool,
            tc.tile_pool(name="psum", bufs=4, space="PSUM") as ppool,
        ):
            w_t = cpool.tile([C_IN, N_HALF, KH * KW, 128], f32r)
            b_t = cpool.tile([128, N_HALF], f32)

            # Bias rides the (otherwise idle) ScalarE HWDGE ring.
            nc.scalar.dma_start(b_t[:], b_d[:])
            # Weights on the Sync HWDGE ring; inputs on the GpSimd SWDGE ring —
            # the two transfer paths run in parallel, so the first matmul group
            # waits on max(w half0, x piece0) instead of their sum. The
            # [c, half, uv, o] host layout keeps each w half contiguous per
            # partition (4608B runs).
            nc.sync.dma_start(w_t[:, 0], w_d[:, 0])

            def load_x(b, first):
                x_t = xpool.tile([C_IN, X_PAD], f32r, tag="x")
                x_flat = x_d[b].rearrange("c h w -> c (h w)")
                for k, (lo, hi) in enumerate(X_PIECES):
                    nc.gpsimd.dma_start(x_t[:, lo:hi], x_flat[:, lo:hi])
                    if first and k == 0:
                        nc.sync.dma_start(w_t[:, 1], w_d[:, 1])
                # Tail pad: (arbitrary) real data — feeds only the invalid
                # output columns (j >= 62) that the host trims away.
                nc.gpsimd.dma_start(
                    x_t[:, H * W : X_PAD], x_flat[:, 0 : X_PAD - H * W]
                )
                return x_t

            for b in range(B_LOC):
                x_t = load_x(b, first=(b == 0))
                for chunk in range(n_chunks):
                    i0 = chunk * ROWS_PER_CHUNK
                    r = min(ROWS_PER_CHUNK, H_OUT - i0)
                    n = r * W
                    p0 = i0 * W
                    for half in range(N_HALF):
                        ps = ppool.tile([128, ROWS_PER_CHUNK, W], f32, tag="ps")
                        for uv in range(KH * KW):
                            u, v = divmod(uv, KW)
                            shift = p0 + u * W + v
                            nc.tensor.matmul(
                                ps[:, 0:r, :],
                                w_t[:, half, uv, :],
                                x_t[:, shift : shift + n],
                                start=(uv == 0),
                                stop=(uv == KH * KW - 1),
                            )
                        o_t = opool.tile([128, ROWS_PER_CHUNK, W], f32, tag="o")
                        nc.vector.tensor_scalar_add(
                            o_t[:, 0:r, :], ps[:, 0:r, :], b_t[:, half : half + 1]
                        )
                        nc.sync.dma_start(
                            y_d[b, half, :, i0 : i0 + r, :], o_t[:, 0:r, :]
                        )

    nc.compile()
    if not nc.is_finalized():
        nc.finalize()
    return nc


def kernel(inputs, weights, bias, profile=False, trace_kwargs=None):
    inputs = np.ascontiguousarray(inputs, dtype=np.float32)
    # [O, C, KH, KW] -> [C, half, KH*KW, o_local]  (lhsT layout: contraction dim
    # on partitions; each half contiguous per partition for fast DMA)
    w_t = np.ascontiguousarray(
        weights.astype(np.float32)
        .reshape(N_HALF, 128, C_IN, KH * KW)
        .transpose(2, 0, 3, 1)
    )
    # [C_OUT, 1] -> [128, N_HALF] with bias_sb[p, h] = bias[h*128 + p]
    b_t = np.ascontiguousarray(
        bias.astype(np.float32).reshape(N_HALF, 128).T
    )

    if "nc" not in _cached:
        _cached["nc"] = _build_nc()
    nc = _cached["nc"]

    in_maps = [
        {
            "x": inputs[i * B_LOC : (i + 1) * B_LOC],
            "w": w_t,
            "bias_in": b_t,
        }
        for i in range(N_CORES)
    ]
    res = run_bass_kernel_spmd(
        nc,
        in_maps,
        list(range(N_CORES)),
        trace=profile,
        **(trace_kwargs or {}),
    )
    _cached["last_result"] = res

    shards = []
    for i in range(N_CORES):
        y = res.results[i]["y"]  # [B_LOC, 2, 128, 62, 64]
        shards.append(y.reshape(B_LOC, C_OUT, H_OUT, W)[..., :W_OUT])
    return np.ascontiguousarray(np.concatenate(shards, axis=0), dtype=np.float32)


# revision 26
# speedup vs baseline: 1.0080x; 1.0080x over previous
"""Trainium2 Bass kernel for 3x3 VALID conv: x[32,128,64,64] * w[256,128,3,3] + bias.

Strategy:
  - Data-parallel over batch: 8 cores x 4 images each; weights/bias replicated.
  - Per core: implicit GEMM. Contraction dim = C_IN = 128 = partition dim.
    For each filter tap (u,v), accumulate
        psum[o, p] += W[c, o; u,v].T @ x[c, p + u*64 + v]
    over the flattened output grid of 62 rows x 64 cols (contiguous moving
    operand; the last 2 cols of each row are invalid and trimmed on host).
  - float32r matmuls (1 cycle/row for free-dim >= 256 vs 4 cycles/row fp32).
  - Critical-path-first DMA: w tap0 + first 17 input rows land first; weights
    stream on the ScalarE HWDGE ring in parallel with inputs on the Sync ring.
  - PSUM -> SBUF evacuation + bias add on VectorE; one output DMA per chunk.
"""

import numpy as np

import concourse.bacc as bacc
import concourse.tile as tile
from concourse import mybir
from concourse.bass_utils import run_bass_kernel_spmd

N_CORES = 8
B_FULL, C_IN, H, W = 32, 128, 64, 64
C_OUT, KH, KW = 256, 3, 3
B_LOC = B_FULL // N_CORES          # images per core
H_OUT, W_OUT = H - KH + 1, W - KW + 1   # 62, 62
N_HALF = C_OUT // 128              # 2 output-channel halves
ROWS_PER_CHUNK = 8                 # 8 out rows x 64 cols = 512 = one PSUM bank
X_PAD = (H_OUT + KH - 1) * W + 128  # padded free size so shifted reads stay in-bounds
X_PIECES = [(0, 656), (656, 1824), (1824, 2976), (2976, H * W)]

_cached = {}


def _build_nc():
    f32 = mybir.dt.float32
    f32r = mybir.dt.float32r
    nc = bacc.Bacc()

    x_d = nc.declare_dram_parameter("x", [B_LOC, C_IN, H, W], f32r, isOutput=False)
    w_d = nc.declare_dram_parameter(
        "w", [C_IN, N_HALF, KH * KW, 128], f32r, isOutput=False
    )
    b_d = nc.declare_dram_parameter("bias_in", [128, N_HALF], f32, isOutput=False)
    y_d = nc.declare_dram_parameter(
        "y", [B_LOC, N_HALF, 128, H_OUT, W], f32, isOutput=True
    )

    n_chunks = (H_OUT + ROWS_PER_CHUNK - 1) // ROWS_PER_CHUNK
    with tile.TileContext(nc) as tc:
        with (
            tc.tile_pool(name="const", bufs=1) as cpool,
            tc.tile_pool(name="xin", bufs=2) as xpool,
            tc.tile_pool(name="out", bufs=4) as opool,
            tc.tile_pool(name="psum", bufs=4, space="PSUM") as ppool,
        ):
            w_t = cpool.tile([C_IN, N_HALF, KH * KW, 128], f32r)
            b_t = cpool.tile([128, N_HALF], f32)

            # Bias rides the (otherwise idle) ScalarE HWDGE ring.
            nc.scalar.dma_start(b_t[:], b_d[:])
            # Critical path for the first matmul group: w half0, then x piece0
            # on the Sync HWDGE ring. The [c, half, uv, o] host layout keeps
            # each w half contiguous per partition (4608B runs).
            nc.sync.dma_start(w_t[:, 0], w_d[:, 0])

            def load_x(b, first):
                x_t = xpool.tile([C_IN, X_PAD], f32r, tag="x")
                x_flat = x_d[b].rearrange("c h w -> c (h w)")
                for k, (lo, hi) in enumerate(X_PIECES):
                    nc.sync.dma_start(x_t[:, lo:hi], x_flat[:, lo:hi])
                    if first and k == 0:
                        nc.sync.dma_start(w_t[:, 1], w_d[:, 1])
                # Tail pad: (arbitrary) real data — feeds only the invalid
                # output columns (j >= 62) that the host trims away.
                nc.sync.dma_start(
                    x_t[:, H * W : X_PAD], x_flat[:, 0 : X_PAD - H * W]
                )
                return x_t

            for b in range(B_LOC):
                x_t = load_x(b, first=(b == 0))
                for chunk in range(n_chunks):
                    i0 = chunk * ROWS_PER_CHUNK
                    r = min(ROWS_PER_CHUNK, H_OUT - i0)
                    n = r * W
                    p0 = i0 * W
                    for half in range(N_HALF):
                        ps = ppool.tile([128, ROWS_PER_CHUNK, W], f32, tag="ps")
                        for uv in range(KH * KW):
                            u, v = divmod(uv, KW)
                            shift = p0 + u * W + v
                            nc.tensor.matmul(
                                ps[:, 0:r, :],
                                w_t[:, half, uv, :],
                                x_t[:, shift : shift + n],
                                start=(uv == 0),
                                stop=(uv == KH * KW - 1),
                            )
                        o_t = opool.tile([128, ROWS_PER_CHUNK, W], f32, tag="o")
                        nc.vector.tensor_scalar_add(
                            o_t[:, 0:r, :], ps[:, 0:r, :], b_t[:, half : half + 1]
                        )
                        nc.sync.dma_start(
                            y_d[b, half, :, i0 : i0 + r, :], o_t[:, 0:r, :]
                        )

    nc.compile()
    if not nc.is_finalized():
        nc.finalize()
    return nc


def kernel(inputs, weights, bias, profile=False, trace_kwargs=None):
    inputs = np.ascontiguousarray(inputs, dtype=np.float32)
    # [O, C, KH, KW] -> [C, half, KH*KW, o_local]  (lhsT layout: contraction dim
    # on partitions; each half contiguous per partition for fast DMA)
    w_t = np.ascontiguousarray(
        weights.astype(np.float32)
        .reshape(N_HALF, 128, C_IN, KH * KW)
        .transpose(2, 0, 3, 1)
    )
    # [C_OUT, 1] -> [128, N_HALF] with bias_sb[p, h] = bias[h*128 + p]
    b_t = np.ascontiguousarray(
        bias.astype(np.float32).reshape(N_HALF, 128).T
    )

    if "nc" not in _cached:
        _cached["nc"] = _build_nc()
    nc = _cached["nc"]

    in_maps = [
        {
            "x": inputs[i * B_LOC : (i + 1) * B_LOC],
            "w": w_t,
            "bias_in": b_t,
        }
        for i in range(N_CORES)
    ]
    res = run_bass_kernel_spmd(
        nc,
        in_maps,
        list(range(N_CORES)),
        trace=profile,
        **(trace_kwargs or {}),
    )
    _cached["last_result"] = res

    shards = []
    for i in range(N_CORES):
        y = res.results[i]["y"]  # [B_LOC, 2, 128, 62, 64]
        shards.append(y.reshape(B_LOC, C_OUT, H_OUT, W)[..., :W_OUT])
    return np.ascontiguousarray(np.concatenate(shards, axis=0), dtype=np.float32)


# revision 27
# speedup vs baseline: 1.0142x; 1.0061x over previous
"""Trainium2 Bass kernel for 3x3 VALID conv: x[32,128,64,64] * w[256,128,3,3] + bias.

Strategy:
  - Data-parallel over batch: 8 cores x 4 images each; weights/bias replicated.
  - Per core: implicit GEMM. Contraction dim = C_IN = 128 = partition dim.
    For each filter tap (u,v), accumulate
        psum[o, p] += W[c, o; u,v].T @ x[c, p + u*64 + v]
    over the flattened output grid of 62 rows x 64 cols (contiguous moving
    operand; the last 2 cols of each row are invalid and trimmed on host).
  - float32r matmuls (1 cycle/row for free-dim >= 256 vs 4 cycles/row fp32).
  - Critical-path-first DMA: w tap0 + first 17 input rows land first; weights
    stream on the ScalarE HWDGE ring in parallel with inputs on the Sync ring.
  - PSUM -> SBUF evacuation + bias add on VectorE; one output DMA per chunk.
"""

import numpy as np

import concourse.bacc as bacc
import concourse.tile as tile
from concourse import mybir
from concourse.bass_utils import run_bass_kernel_spmd

N_CORES = 8
B_FULL, C_IN, H, W = 32, 128, 64, 64
C_OUT, KH, KW = 256, 3, 3
B_LOC = B_FULL // N_CORES          # images per core
H_OUT, W_OUT = H - KH + 1, W - KW + 1   # 62, 62
N_HALF = C_OUT // 128              # 2 output-channel halves
ROWS_PER_CHUNK = 8                 # 8 out rows x 64 cols = 512 = one PSUM bank
X_PAD = (H_OUT + KH - 1) * W + 128  # padded free size so shifted reads stay in-bounds
X_PIECES = [(0, 656), (656, 1824), (1824, 2976), (2976, H * W)]

_cached = {}


def _build_nc():
    f32 = mybir.dt.float32
    f32r = mybir.dt.float32r
    nc = bacc.Bacc()

    x_d = nc.declare_dram_parameter("x", [B_LOC, C_IN, H, W], f32r, isOutput=False)
    w_d = nc.declare_dram_parameter(
        "w", [C_IN, N_HALF, KH * KW, 128], f32r, isOutput=False
    )
    b_d = nc.declare_dram_parameter("bias_in", [128, N_HALF], f32, isOutput=False)
    y_d = nc.declare_dram_parameter(
        "y", [B_LOC, N_HALF, 128, H_OUT, W], f32, isOutput=True
    )

    n_chunks = (H_OUT + ROWS_PER_CHUNK - 1) // ROWS_PER_CHUNK
    with tile.TileContext(nc) as tc:
        with (
            tc.tile_pool(name="const", bufs=1) as cpool,
            tc.tile_pool(name="xin", bufs=2) as xpool,
            tc.tile_pool(name="out", bufs=4) as opool,
            tc.tile_pool(name="psum", bufs=4, space="PSUM") as ppool,
        ):
            w_t = cpool.tile([C_IN, N_HALF, KH * KW, 128], f32r)
            b_t = cpool.tile([128, N_HALF], f32)

            # Bias rides the (otherwise idle) ScalarE HWDGE ring.
            nc.scalar.dma_start(b_t[:], b_d[:])
            # Critical path for the first matmul group: w half0 taps 0-2, then
            # x piece0 on the Sync HWDGE ring. The [c, half, uv, o] host layout
            # keeps every tap subrange contiguous per partition, so the first
            # three matmuls can start while the remaining taps stream in.
            nc.sync.dma_start(w_t[:, 0, 0:3], w_d[:, 0, 0:3])

            def load_x(b, first):
                x_t = xpool.tile([C_IN, X_PAD], f32r, tag="x")
                x_flat = x_d[b].rearrange("c h w -> c (h w)")
                for k, (lo, hi) in enumerate(X_PIECES):
                    nc.sync.dma_start(x_t[:, lo:hi], x_flat[:, lo:hi])
                    if first and k == 0:
                        nc.sync.dma_start(w_t[:, 0, 3 : KH * KW], w_d[:, 0, 3 : KH * KW])
                        nc.sync.dma_start(w_t[:, 1], w_d[:, 1])
                # Tail pad: (arbitrary) real data — feeds only the invalid
                # output columns (j >= 62) that the host trims away.
                nc.sync.dma_start(
                    x_t[:, H * W : X_PAD], x_flat[:, 0 : X_PAD - H * W]
                )
                return x_t

            for b in range(B_LOC):
                x_t = load_x(b, first=(b == 0))
                for chunk in range(n_chunks):
                    i0 = chunk * ROWS_PER_CHUNK
                    r = min(ROWS_PER_CHUNK, H_OUT - i0)
                    n = r * W
                    p0 = i0 * W
                    for half in range(N_HALF):
                        ps = ppool.tile([128, ROWS_PER_CHUNK, W], f32, tag="ps")
                        for uv in range(KH * KW):
                            u, v = divmod(uv, KW)
                            shift = p0 + u * W + v
                            nc.tensor.matmul(
                                ps[:, 0:r, :],
                                w_t[:, half, uv, :],
                                x_t[:, shift : shift + n],
                                start=(uv == 0),
                                stop=(uv == KH * KW - 1),
                            )
                        o_t = opool.tile([128, ROWS_PER_CHUNK, W], f32, tag="o")
                        nc.vector.tensor_scalar_add(
                            o_t[:, 0:r, :], ps[:, 0:r, :], b_t[:, half : half + 1]
                        )
                        nc.sync.dma_start(
                            y_d[b, half, :, i0 : i0 + r, :], o_t[:, 0:r, :]
                        )

    nc.compile()
    if not nc.is_finalized():
        nc.finalize()
    return nc


def kernel(inputs, weights, bias, profile=False, trace_kwargs=None):
    inputs = np.ascontiguousarray(inputs, dtype=np.float32)
    # [O, C, KH, KW] -> [C, half, KH*KW, o_local]  (lhsT layout: contraction dim
    # on partitions; each half contiguous per partition for fast DMA)
    w_t = np.ascontiguousarray(
        weights.astype(np.float32)
        .reshape(N_HALF, 128, C_IN, KH * KW)
        .transpose(2, 0, 3, 1)
    )
    # [C_OUT, 1] -> [128, N_HALF] with bias_sb[p, h] = bias[h*128 + p]
    b_t = np.ascontiguousarray(
        bias.astype(np.float32).reshape(N_HALF, 128).T
    )

    if "nc" not in _cached:
        _cached["nc"] = _build_nc()
    nc = _cached["nc"]

    in_maps = [
        {
            "x": inputs[i * B_LOC : (i + 1) * B_LOC],
            "w": w_t,
            "bias_in": b_t,
        }
        for i in range(N_CORES)
    ]
    res = run_bass_kernel_spmd(
        nc,
        in_maps,
        list(range(N_CORES)),
        trace=profile,
        **(trace_kwargs or {}),
    )
    _cached["last_result"] = res

    shards = []
    for i in range(N_CORES):
        y = res.results[i]["y"]  # [B_LOC, 2, 128, 62, 64]
        shards.append(y.reshape(B_LOC, C_OUT, H_OUT, W)[..., :W_OUT])
    return np.ascontiguousarray(np.concatenate(shards, axis=0), dtype=np.float32)


# revision 29
# speedup vs baseline: 1.0179x; 1.0037x over previous
"""Trainium2 Bass kernel for 3x3 VALID conv: x[32,128,64,64] * w[256,128,3,3] + bias.

Strategy:
  - Data-parallel over batch: 8 cores x 4 images each; weights/bias replicated.
  - Per core: implicit GEMM. Contraction dim = C_IN = 128 = partition dim.
    For each filter tap (u,v), accumulate
        psum[o, p] += W[c, o; u,v].T @ x[c, p + u*64 + v]
    over the flattened output grid of 62 rows x 64 cols (contiguous moving
    operand; the last 2 cols of each row are invalid and trimmed on host).
  - float32r matmuls (1 cycle/row for free-dim >= 256 vs 4 cycles/row fp32).
  - Critical-path-first DMA: w tap0 + first 17 input rows land first; weights
    stream on the ScalarE HWDGE ring in parallel with inputs on the Sync ring.
  - PSUM -> SBUF evacuation + bias add on VectorE; one output DMA per chunk.
"""

import numpy as np

import concourse.bacc as bacc
import concourse.tile as tile
from concourse import mybir
from concourse.bass_utils import run_bass_kernel_spmd

N_CORES = 8
B_FULL, C_IN, H, W = 32, 128, 64, 64
C_OUT, KH, KW = 256, 3, 3
B_LOC = B_FULL // N_CORES          # images per core
H_OUT, W_OUT = H - KH + 1, W - KW + 1   # 62, 62
N_HALF = C_OUT // 128              # 2 output-channel halves
ROWS_PER_CHUNK = 8                 # 8 out rows x 64 cols = 512 = one PSUM bank
X_PAD = (H_OUT + KH - 1) * W + 128  # padded free size so shifted reads stay in-bounds
X_PIECES = [(0, 656), (656, 1824), (1824, 2976), (2976, H * W)]

_cached = {}


def _build_nc():
    f32 = mybir.dt.float32
    f32r = mybir.dt.float32r
    nc = bacc.Bacc()

    x_d = nc.declare_dram_parameter("x", [B_LOC, C_IN, H, W], f32r, isOutput=False)
    w_d = nc.declare_dram_parameter(
        "w", [C_IN, N_HALF, KH * KW, 128], f32r, isOutput=False
    )
    b_d = nc.declare_dram_parameter("bias_in", [128, N_HALF], f32, isOutput=False)
    y_d = nc.declare_dram_parameter(
        "y", [B_LOC, N_HALF, 128, H_OUT, W], f32, isOutput=True
    )

    n_chunks = (H_OUT + ROWS_PER_CHUNK - 1) // ROWS_PER_CHUNK
    with tile.TileContext(nc) as tc:
        with (
            tc.tile_pool(name="const", bufs=1) as cpool,
            tc.tile_pool(name="xin", bufs=2) as xpool,
            tc.tile_pool(name="out", bufs=4) as opool,
            tc.tile_pool(name="psum", bufs=4, space="PSUM") as ppool,
        ):
            w_t = cpool.tile([C_IN, N_HALF, KH * KW, 128], f32r)
            b_t = cpool.tile([128, N_HALF], f32)

            # Bias rides the (otherwise idle) ScalarE HWDGE ring.
            nc.scalar.dma_start(b_t[:], b_d[:])
            # Critical path for the first matmul group: w half0 taps 0-2, then
            # x piece0 on the Sync HWDGE ring. The [c, half, uv, o] host layout
            # keeps every tap subrange contiguous per partition, so the first
            # three matmuls can start while the remaining taps stream in.
            nc.sync.dma_start(w_t[:, 0, 0:3], w_d[:, 0, 0:3])

            def load_x(b, first):
                x_t = xpool.tile([C_IN, X_PAD], f32r, tag="x")
                x_flat = x_d[b].rearrange("c h w -> c (h w)")
                for k, (lo, hi) in enumerate(X_PIECES):
                    nc.sync.dma_start(x_t[:, lo:hi], x_flat[:, lo:hi])
                    if first and k == 0:
                        nc.sync.dma_start(w_t[:, 0, 3 : KH * KW], w_d[:, 0, 3 : KH * KW])
                        nc.sync.dma_start(w_t[:, 1], w_d[:, 1])
                # Tail pad: (arbitrary) real data — feeds only the invalid
                # output columns (j >= 62) that the host trims away.
                nc.sync.dma_start(
                    x_t[:, H * W : X_PAD], x_flat[:, 0 : X_PAD - H * W]
                )
                return x_t

            for b in range(B_LOC):
                x_t = load_x(b, first=(b == 0))
                for chunk in range(n_chunks):
                    i0 = chunk * ROWS_PER_CHUNK
                    r = min(ROWS_PER_CHUNK, H_OUT - i0)
                    n = r * W
                    p0 = i0 * W
                    for half in range(N_HALF):
                        ps = ppool.tile([128, ROWS_PER_CHUNK, W], f32, tag="ps")
                        for uv in range(KH * KW):
                            u, v = divmod(uv, KW)
                            shift = p0 + u * W + v
                            nc.tensor.matmul(
                                ps[:, 0:r, :],
                                w_t[:, half, uv, :],
                                x_t[:, shift : shift + n],
                                start=(uv == 0),
                                stop=(uv == KH * KW - 1),
                            )
                        o_t = opool.tile([128, ROWS_PER_CHUNK, W], f32, tag="o")
                        nc.vector.tensor_scalar_add(
                            o_t[:, 0:r, :], ps[:, 0:r, :], b_t[:, half : half + 1]
                        )
                        nc.sync.dma_start(
                            y_d[b, half, :, i0 : i0 + r, :], o_t[:, 0:r, :]
                        )

    nc.compile()
    if not nc.is_finalized():
        nc.finalize()
    return nc


def kernel(inputs, weights, bias, profile=False, trace_kwargs=None):
    inputs = np.ascontiguousarray(inputs, dtype=np.float32)
    # [O, C, KH, KW] -> [C, half, KH*KW, o_local]  (lhsT layout: contraction dim
    # on partitions; each half contiguous per partition for fast DMA)
    w_t = np.ascontiguousarray(
        weights.astype(np.float32)
        .reshape(N_HALF, 128, C_IN, KH * KW)
        .transpose(2, 0, 3, 1)
    )
    # [C_OUT, 1] -> [128, N_HALF] with bias_sb[p, h] = bias[h*128 + p]
    b_t = np.ascontiguousarray(
        bias.astype(np.float32).reshape(N_HALF, 128).T
    )

    if "nc" not in _cached:
        _cached["nc"] = _build_nc()
    nc = _cached["nc"]

    in_maps = [
        {
            "x": inputs[i * B_LOC : (i + 1) * B_LOC],
            "w": w_t,
            "bias_in": b_t,
        }
        for i in range(N_CORES)
    ]
    res = run_bass_kernel_spmd(
        nc,
        in_maps,
        list(range(N_CORES)),
        trace=profile,
        **(trace_kwargs or {}),
    )
    _cached["last_result"] = res

    shards = []
    for i in range(N_CORES):
        y = res.results[i]["y"]  # [B_LOC, 2, 128, 62, 64]
        shards.append(y.reshape(B_LOC, C_OUT, H_OUT, W)[..., :W_OUT])
    return np.ascontiguousarray(np.concatenate(shards, axis=0), dtype=np.float32)
